# revision 13
# baseline (speedup 1.0000x reference)
"""Trainium2 Bass kernel for nn_Encoder_37340445671714 (video ViT encoder).

Sharding: 8 cores = 4 batch elements x 2 sequence halves (788 tokens each).
Each core runs the full 6-layer encoder for its (batch, half):
  - activations kept transposed [feature, token] in SBUF
  - all matmuls bf16 (fp32 PSUM accumulate), residual stream bf16
  - flash-style attention: scores^T per k-tile -> Exp on ScalarE -> AV
    accumulation with 64 ones-columns in V replicating the softmax
    denominator Z across PSUM partitions 64:128 (normalization via
    exp(-ln Z) on ScalarE)
  - LayerNorm stats via ones-matmul partition sums on TensorE
  - per-layer pair AllGather (bf16) exchanges the updated half sequence
Weights are pre-transposed/padded on the host (free) into matmul-ready
layouts. Output is transposed back to natural layout on the PE at the end.
"""

import numpy as np
import ml_dtypes

import concourse.bass as bass
import concourse.tile as tile
from concourse import mybir
from concourse.bass_utils import run_bass_kernel_spmd

F32 = mybir.dt.float32
F32R = mybir.dt.float32r
BF16 = mybir.dt.bfloat16
AF = mybir.ActivationFunctionType
OP = mybir.AluOpType

# problem dims
B, L, C, H, W = 4, 8, 3, 224, 224
PH = PW = 16
D = 512
NH = 8
DK = 64
FF = 2048
NL = 6
NP = (H // PH) * (W // PW)  # 196
S = L * (NP + 1)  # 1576
PD = PH * PW * C  # 768
OWN = S // 2  # 788 tokens per core
LN_EPS = 1e-5

DC = D // 128  # 4 feature subtiles
PDC = PD // 128  # 6
FTC = FF // 128  # 16

# q chunks (the 2 halves of the own-token range)
QC = [(0, 394), (394, 394)]
# k tiles over the full sequence
KT = [(i * 128, 128) for i in range(S // 128)] + [(S - S % 128, S % 128)]  # 12x128+40

N_CORES = 8
REPLICA_GROUPS = [[0, 1], [2, 3], [4, 5], [6, 7]]


def legalize_waits(nc):
    """Split multi-wait instructions into preceding single-wait NoOps.

    The walrus build in this environment rejects instructions carrying more
    than one semaphore wait command.
    """
    n_split = 0
    for f in nc.m.functions:
        for bb in f.blocks:
            insts = list(bb.instructions)
            new_insts = []
            changed = False
            for inst in insts:
                si = inst.sync_info
                if si is not None and len(si.on_wait) > 1:
                    waits = list(si.on_wait)
                    for w in waits[:-1]:
                        nop = mybir.InstNoOp(
                            name=nc.get_next_instruction_name(),
                            engine=inst.engine,
                            ins=[],
                            outs=[],
                        )
                        nop.sync_info = mybir.SyncInfo(on_wait=[w], on_update=[])
                        new_insts.append(nop)
                        n_split += 1
                    inst.sync_info = mybir.SyncInfo(
                        on_wait=[waits[-1]], on_update=list(si.on_update)
                    )
                    changed = True
                new_insts.append(inst)
            if changed:
                bb.instructions = new_insts
    return n_split


def _bcast_ap(ap_1d, parts=128):
    """Partition-broadcast DRAM AP: [n] -> [parts, n] with partition stride 0."""
    return bass.AP(
        tensor=ap_1d.tensor, offset=ap_1d.offset, ap=[[0, parts]] + list(ap_1d.ap)
    )


def build_kernel():
    nc = bass.Bass(
        "TRN2", target_bir_lowering=False, debug=False, num_devices=N_CORES
    )

    # ---- I/O ----
    pat = nc.dram_tensor("pat", [PD, OWN], BF16, kind="ExternalInput").ap()
    addv = nc.dram_tensor("addv", [D, OWN], F32, kind="ExternalInput").ap()
    wembT = nc.dram_tensor("wembT", [PD, D], BF16, kind="ExternalInput").ap()
    wqT = nc.dram_tensor("wqT", [NL, D, D], BF16, kind="ExternalInput").ap()
    wkT = nc.dram_tensor("wkT", [NL, D, D], BF16, kind="ExternalInput").ap()
    wvT = nc.dram_tensor("wvT", [NL, D, D], BF16, kind="ExternalInput").ap()
    woT = nc.dram_tensor("woT", [NL, D, D], BF16, kind="ExternalInput").ap()
    w1T = nc.dram_tensor("w1T", [NL, D, FF], BF16, kind="ExternalInput").ap()
    w2T = nc.dram_tensor("w2T", [NL, FF, D], BF16, kind="ExternalInput").ap()
    bq = nc.dram_tensor("bq", [NL, D], F32, kind="ExternalInput").ap()
    bk = nc.dram_tensor("bk", [NL, D], F32, kind="ExternalInput").ap()
    bv = nc.dram_tensor("bv", [NL, D], F32, kind="ExternalInput").ap()
    bo = nc.dram_tensor("bo", [NL, D], F32, kind="ExternalInput").ap()
    b1 = nc.dram_tensor("b1", [NL, FF], F32, kind="ExternalInput").ap()
    b2 = nc.dram_tensor("b2", [NL, D], F32, kind="ExternalInput").ap()
    g1 = nc.dram_tensor("g1", [NL, D], F32, kind="ExternalInput").ap()
    be1 = nc.dram_tensor("be1", [NL, D], F32, kind="ExternalInput").ap()
    g2 = nc.dram_tensor("g2", [NL, D], F32, kind="ExternalInput").ap()
    be2 = nc.dram_tensor("be2", [NL, D], F32, kind="ExternalInput").ap()
    ident = nc.dram_tensor("ident", [128, 128], BF16, kind="ExternalInput").ap()
    xout = nc.dram_tensor("xout", [OWN, D], F32, kind="ExternalOutput").ap()

    with tile.TileContext(nc) as tc:
        with (
            tc.tile_pool(name="const", bufs=1) as constp,
            tc.tile_pool(name="wsmall", bufs=1) as wsmall,
            tc.tile_pool(name="wff", bufs=3) as wff,
            tc.tile_pool(name="xp", bufs=2) as xp,
            tc.tile_pool(name="kv", bufs=2) as kvp,
            tc.tile_pool(name="vp", bufs=1) as vp,
            tc.tile_pool(name="qo", bufs=1) as qop,
            tc.tile_pool(name="zp", bufs=1) as zp,
            tc.tile_pool(name="zbp", bufs=1) as zbp,
            tc.tile_pool(name="big", bufs=1) as bigp,
            tc.tile_pool(name="exps", bufs=3) as expp,
            tc.tile_pool(name="stat", bufs=6) as statp,
            tc.tile_pool(name="rz", bufs=3) as rzp,
            tc.tile_pool(name="bias", bufs=2) as biasp,
            tc.tile_pool(name="psA", bufs=1, space="PSUM") as psA,
            tc.tile_pool(name="psB", bufs=2, space="PSUM") as psB,
            tc.tile_pool(name="dram", bufs=2, space="DRAM") as dramp,
        ):
            P = dict(
                constp=constp, wsmall=wsmall, wff=wff, xp=xp, kvp=kvp, vp=vp,
                qop=qop, zp=zp, zbp=zbp, bigp=bigp, expp=expp, statp=statp,
                rzp=rzp, biasp=biasp, psA=psA, psB=psB, dramp=dramp,
            )
            dram_in = dict(
                pat=pat, addv=addv, wembT=wembT, wqT=wqT, wkT=wkT, wvT=wvT,
                woT=woT, w1T=w1T, w2T=w2T, bq=bq, bk=bk, bv=bv, bo=bo, b1=b1,
                b2=b2, g1=g1, be1=be1, g2=g2, be2=be2, ident=ident, xout=xout,
            )
            ones_bf = constp.tile([128, 128], BF16, name="ones_bf")
            nc.vector.memset(ones_bf[:], 1.0)
            ones_r = constp.tile([128, 128], F32R, name="ones_r")
            nc.vector.tensor_scalar_add(ones_r[:], ones_bf[:], 0.0)
            ident_sb = constp.tile([128, 128], BF16, name="ident_sb")
            nc.sync.dma_start(ident_sb[:], ident[:])
            eps_sb = constp.tile([128, 1], F32, name="eps_sb")
            nc.vector.memset(eps_sb[:], LN_EPS)
            P["ones_bf"] = ones_bf
            P["ones_r"] = ones_r
            P["ident_sb"] = ident_sb
            P["eps_sb"] = eps_sb

            x_bf = _embed(nc, P, dram_in)
            for l in range(NL):
                x_bf = _one_layer(nc, P, dram_in, l, x_bf)
            _tail(nc, P, dram_in, x_bf)
    return nc


def _embed(nc, P, dr):
    bigp, zp, wff, xp, psB = P["bigp"], P["zp"], P["wff"], P["xp"], P["psB"]
    pat_sb = bigp.tile([128, FTC, OWN], BF16, tag="h", name="pat_sb")
    nc.sync.dma_start(
        pat_sb[:, :PDC, :], dr["pat"].rearrange("(ko p) t -> p ko t", p=128)
    )
    addv_sb = zp.tile([128, DC, OWN], F32, tag="z", name="addv_sb")
    nc.sync.dma_start(addv_sb[:], dr["addv"].rearrange("(co p) t -> p co t", p=128))
    wemb_sb = wff.tile([128, PDC, D], BF16, tag="wff", name="wemb_sb")
    nc.sync.dma_start(wemb_sb[:], dr["wembT"].rearrange("(ko p) d -> p ko d", p=128))

    x_bf = xp.tile([128, DC, OWN], BF16, tag="x", name="x_emb")
    for dt in range(DC):
        for (q0, qn) in QC:
            ps = psB.tile([128, 2, 512], F32, tag="s", name="ps_emb")
            for kt in range(PDC):
                nc.tensor.matmul(
                    ps[:, 0, :qn],
                    wemb_sb[:, kt, dt * 128 : (dt + 1) * 128],
                    pat_sb[:, kt, q0 : q0 + qn],
                    start=(kt == 0),
                    stop=(kt == PDC - 1),
                )
            nc.vector.tensor_tensor(
                x_bf[:, dt, q0 : q0 + qn],
                ps[:, 0, :qn],
                addv_sb[:, dt, q0 : q0 + qn],
                OP.add,
            )
    return x_bf


def _load_layer_params(nc, P, dr, l):
    biasp, wsmall = P["biasp"], P["wsmall"]
    prm = {}
    for nm in ["bq", "bk", "bo", "b2", "g1", "be1", "g2", "be2"]:
        t = biasp.tile([128, DC], F32, tag=nm, name=nm + "_sb")
        nc.sync.dma_start(t[:], dr[nm][l].rearrange("(o p) -> p o", p=128))
        prm[nm] = t
    b1_sb = biasp.tile([128, FTC], F32, tag="b1", name="b1_sb")
    nc.sync.dma_start(b1_sb[:], dr["b1"][l].rearrange("(o p) -> p o", p=128))
    prm["b1"] = b1_sb
    bv_bc = biasp.tile([128, D], F32, tag="bvb", name="bv_bc")
    nc.gpsimd.dma_start(bv_bc[:], _bcast_ap(dr["bv"][l]))
    prm["bv_bc"] = bv_bc
    for nm, key in [("wq", "wqT"), ("wk", "wkT"), ("wv", "wvT"), ("wo", "woT")]:
        t = wsmall.tile([128, DC, D], BF16, tag=nm, name=nm + "_sb")
        nc.sync.dma_start(t[:], dr[key][l].rearrange("(co p) d -> p co d", p=128))
        prm[nm] = t
    return prm


def _allgather_x(nc, P, x_bf):
    dramp, kvp = P["dramp"], P["kvp"]
    xg_in = dramp.tile([DC, 128, OWN], BF16, tag="agi", name="xg_in")
    nc.sync.dma_start(xg_in[:].rearrange("c p t -> p c t"), x_bf[:])
    xg_out = dramp.tile([2, DC, 128, OWN], BF16, tag="ago", name="xg_out")
    nc.gpsimd.collective_compute(
        "AllGather",
        OP.bypass,
        replica_groups=REPLICA_GROUPS,
        ins=[xg_in[:].opt()],
        outs=[xg_out[:].opt()],
    )
    x_all = kvp.tile([128, DC, S], BF16, tag="kv", name="x_all")
    for s_ in range(2):
        nc.sync.dma_start(
            x_all[:, :, s_ * OWN : (s_ + 1) * OWN],
            xg_out[s_].rearrange("c p t -> p c t"),
        )
    return x_all


def _proj_to_T(nc, psB, w_sb, rhs, out_sb, bias_sb, chunks, n_ct=DC):
    """out_sb[:, dt, chunk] (bf16, transposed) = w^T-style proj + per-partition bias."""
    for dt in range(DC):
        for (t0, tn) in chunks:
            ps = psB.tile([128, 2, 512], F32, tag="s", name="ps_p")
            for ct in range(n_ct):
                nc.tensor.matmul(
                    ps[:, 0, :tn],
                    w_sb[:, ct, dt * 128 : (dt + 1) * 128],
                    rhs[:, ct, t0 : t0 + tn],
                    start=(ct == 0),
                    stop=(ct == n_ct - 1),
                )
            nc.vector.tensor_scalar_add(
                out_sb[:, dt, t0 : t0 + tn], ps[:, 0, :tn], bias_sb[:, dt : dt + 1]
            )


def _attention(nc, P, prm, x_bf, x_all):
    psA, psB, qop, vp, expp, rzp = (
        P["psA"], P["psB"], P["qop"], P["vp"], P["expp"], P["rzp"]
    )
    ones_bf = P["ones_bf"]
    qT = qop.tile([128, DC, OWN], BF16, tag="qT", name="qT")
    _proj_to_T(nc, psB, prm["wq"], x_bf, qT, prm["bq"], QC)
    kT = P["kvp"].tile([128, DC, S], BF16, tag="kv", name="kT")
    _proj_to_T(
        nc, psB, prm["wk"], x_all, kT, prm["bk"], [(i * 394, 394) for i in range(4)]
    )

    v_bf = vp.tile([128, len(KT), NH, 64], BF16, tag="v", name="v_bf")
    for ti, (k0, ksz) in enumerate(KT):
        ps = psB.tile([128, 2, 512], F32, tag="s", name="ps_v")
        for ct in range(DC):
            nc.tensor.matmul(
                ps[:ksz, 0, :D],
                x_all[:, ct, k0 : k0 + ksz],
                prm["wv"][:, ct, :],
                start=(ct == 0),
                stop=(ct == DC - 1),
            )
        nc.vector.tensor_tensor(
            v_bf[:ksz, ti, :, :],
            ps[:ksz, 0, :D].rearrange("t (h e) -> t h e", h=NH),
            prm["bv_bc"][:ksz, :].rearrange("t (h e) -> t h e", h=NH),
            OP.add,
        )

    oT = qop.tile([128, DC, OWN], BF16, tag="oT", name="oT")
    for hpair in range(NH // 2):
        hdt = hpair
        o_ps = psA.tile([128, 2, 512], F32, tag="o", name="o_ps")
        z_ps = psA.tile([128, 2, 512], F32, tag="zz", name="z_ps")
        for ti, (k0, ksz) in enumerate(KT):
            first, last = (ti == 0), (ti == len(KT) - 1)
            for sub in range(2):
                hp = sub * 64
                s_ps = psB.tile([128, 2, 512], F32, tag="s", name="s_ps")
                for ci, (q0, qn) in enumerate(QC):
                    nc.tensor.matmul(
                        s_ps[:ksz, ci, :qn],
                        kT[hp : hp + 64, hdt, k0 : k0 + ksz],
                        qT[hp : hp + 64, hdt, q0 : q0 + qn],
                        start=True,
                        stop=True,
                    )
                e_sb = expp.tile([128, 2, 394], BF16, tag="e", name="expS")
                nc.scalar.activation(
                    e_sb[:ksz, :, :], s_ps[:ksz, :, 0:394], AF.Exp, scale=0.125
                )
                for ci, (q0, qn) in enumerate(QC):
                    nc.tensor.matmul(
                        o_ps[hp : hp + 64, ci, :qn],
                        v_bf[:ksz, ti, 2 * hpair + sub, :],
                        e_sb[:ksz, ci, :qn],
                        start=first,
                        stop=last,
                        tile_position=(0, hp),
                    )
                    nc.tensor.matmul(
                        z_ps[hp : hp + 64, ci, :qn],
                        ones_bf[:ksz, 0:64],
                        e_sb[:ksz, ci, :qn],
                        start=first,
                        stop=last,
                        tile_position=(0, hp),
                    )
        lnz = rzp.tile([128, 2, 394], F32, tag="rz", name="lnz")
        nc.scalar.activation(lnz[:], z_ps[:, :, 0:394], AF.Ln)
        rzb = rzp.tile([128, 2, 394], F32, tag="rz", name="rzb")
        nc.scalar.activation(rzb[:], lnz[:], AF.Exp, scale=-1.0)
        nc.vector.tensor_tensor(
            oT[:, hdt, :].rearrange("p (a b) -> p a b", a=2),
            o_ps[:, :, 0:394],
            rzb[:],
            OP.mult,
        )
    return oT


def _residual_proj(nc, psB, w_sb, rhs_T, bias_sb, x_bf, z_out):
    """z_out (f32) = W^T proj of rhs_T + bias + x_bf (residual)."""
    for dt in range(DC):
        for (q0, qn) in QC:
            ps = psB.tile([128, 2, 512], F32, tag="s", name="ps_r")
            for ct in range(DC):
                nc.tensor.matmul(
                    ps[:, 0, :qn],
                    w_sb[:, ct, dt * 128 : (dt + 1) * 128],
                    rhs_T[:, ct, q0 : q0 + qn],
                    start=(ct == 0),
                    stop=(ct == DC - 1),
                )
            nc.vector.scalar_tensor_tensor(
                z_out[:, dt, q0 : q0 + qn],
                ps[:, 0, :qn],
                bias_sb[:, dt : dt + 1],
                x_bf[:, dt, q0 : q0 + qn],
                OP.add,
                OP.add,
            )


def _ffn(nc, P, dr, l, prm, x_bf, z_out):
    psA, psB, bigp, wff = P["psA"], P["psB"], P["bigp"], P["wff"]
    h_bf = bigp.tile([128, FTC, OWN], BF16, tag="h", name="h_bf")
    for half in range(2):
        w1_sb = wff.tile([128, DC, FF // 2], BF16, tag="wff", name="w1_sb")
        nc.sync.dma_start(
            w1_sb[:],
            dr["w1T"][l][:, half * (FF // 2) : (half + 1) * (FF // 2)].rearrange(
                "(co p) f -> p co f", p=128
            ),
        )
        for j in range(FTC // 2):
            ft = half * (FTC // 2) + j
            ps = psB.tile([128, 2, 512], F32, tag="s", name="ps_h")
            for ci, (q0, qn) in enumerate(QC):
                for ct in range(DC):
                    nc.tensor.matmul(
                        ps[:, ci, :qn],
                        w1_sb[:, ct, j * 128 : (j + 1) * 128],
                        x_bf[:, ct, q0 : q0 + qn],
                        start=(ct == 0),
                        stop=(ct == DC - 1),
                    )
            nc.vector.tensor_scalar(
                h_bf[:, ft, :].rearrange("p (a b) -> p a b", a=2),
                ps[:, :, 0:394],
                prm["b1"][:, ft : ft + 1],
                0.0,
                OP.add,
                OP.max,
            )

    w2_halves = []
    for half in range(2):
        w2_sb = wff.tile([128, FTC // 2, D], BF16, tag="wff", name="w2_sb")
        nc.sync.dma_start(
            w2_sb[:],
            dr["w2T"][l][half * (FF // 2) : (half + 1) * (FF // 2), :].rearrange(
                "(fo p) d -> p fo d", p=128
            ),
        )
        w2_halves.append(w2_sb)
    for dt in range(DC):
        for (q0, qn) in QC:
            ps2 = psA.tile([128, 2, 512], F32, tag="o", name="ps_f")
            for half in range(2):
                for j in range(FTC // 2):
                    ft = half * (FTC // 2) + j
                    nc.tensor.matmul(
                        ps2[:, 0, :qn],
                        w2_halves[half][:, j, dt * 128 : (dt + 1) * 128],
                        h_bf[:, ft, q0 : q0 + qn],
                        start=(ft == 0),
                        stop=(ft == FTC - 1),
                    )
            nc.vector.scalar_tensor_tensor(
                z_out[:, dt, q0 : q0 + qn],
                ps2[:, 0, :qn],
                prm["b2"][:, dt : dt + 1],
                x_bf[:, dt, q0 : q0 + qn],
                OP.add,
                OP.add,
            )


def _one_layer(nc, P, dr, l, x_bf):
    prm = _load_layer_params(nc, P, dr, l)
    x_all = _allgather_x(nc, P, x_bf)
    oT = _attention(nc, P, prm, x_bf, x_all)
    z = P["zp"].tile([128, DC, OWN], F32R, tag="z", name=f"z1_{l}")
    _residual_proj(nc, P["psB"], prm["wo"], oT, prm["bo"], x_bf, z)
    x_bf = P["xp"].tile([128, DC, OWN], BF16, tag="x", name=f"x_ln1_{l}")
    _layernorm(nc, P, z, x_bf, prm["g1"], prm["be1"])
    z = P["zp"].tile([128, DC, OWN], F32R, tag="z", name=f"z2_{l}")
    _ffn(nc, P, dr, l, prm, x_bf, z)
    last = l == NL - 1
    if last:
        x2 = P["bigp"].tile([128, DC, OWN], F32, tag="h", name="x_final")
    else:
        x2 = P["xp"].tile([128, DC, OWN], BF16, tag="x", name=f"x_ln2_{l}")
    _layernorm(nc, P, z, x2, prm["g2"], prm["be2"])
    return x2


def _tail(nc, P, dr, x_f32):
    psB, bigp = P["psB"], P["bigp"]
    xout = dr["xout"]
    ident32 = P["constp"].tile([128, 128], F32, name="ident32")
    nc.vector.tensor_scalar_add(ident32[:], P["ident_sb"][:], 0.0)
    for ti in range(7):
        t0 = ti * 128
        tsz = min(128, OWN - t0)
        xo_sb = P["statp"].tile([128, D], F32, tag="st", name="xo_sb")
        for dt in range(DC):
            tp = psB.tile([128, 2, 512], F32, tag="s", name="tp")
            nc.tensor.transpose(
                tp[:tsz, 0, :128], x_f32[:, dt, t0 : t0 + tsz], ident32[:]
            )
            nc.vector.tensor_scalar_add(
                xo_sb[:tsz, dt * 128 : (dt + 1) * 128], tp[:tsz, 0, :128], 0.0
            )
        nc.sync.dma_start(xout[t0 : t0 + tsz, :], xo_sb[:tsz, :])


def _layernorm(nc, P, z, x_out, g_sb, be_sb):
    """Post-LN over features (partition dim) in transposed layout.

    z: [128, DC, OWN] f32r.  Writes x_out = (z - mu) * rstd * g + b.
    Stats are computed with f32r ones-matmuls directly on z (full precision
    vs the bf16 shadow it replaces)."""
    psB, statp, zbp = P["psB"], P["statp"], P["zbp"]
    ones_r = P["ones_r"]
    zf = z[:].bitcast(F32)
    sum_ps = psB.tile([128, 2, 512], F32, tag="s", name="sum_ps")
    for ci, (q0, qn) in enumerate(QC):
        for ct in range(DC):
            nc.tensor.matmul(
                sum_ps[:, ci, :qn],
                ones_r[:],
                z[:, ct, q0 : q0 + qn],
                start=(ct == 0),
                stop=(ct == DC - 1),
            )
    sq = zbp.tile([128, DC, OWN], F32R, tag="zb", name="sq_r")
    nc.vector.tensor_tensor(sq[:], zf, zf, OP.mult)
    sq_ps = psB.tile([128, 2, 512], F32, tag="s", name="sq_ps")
    for ci, (q0, qn) in enumerate(QC):
        for ct in range(DC):
            nc.tensor.matmul(
                sq_ps[:, ci, :qn],
                ones_r[:],
                sq[:, ct, q0 : q0 + qn],
                start=(ct == 0),
                stop=(ct == DC - 1),
            )
    mu = statp.tile([128, 2, 394], F32, tag="st", name="mu")
    nc.vector.tensor_scalar(
        mu[:], sum_ps[:, :, 0:394], 1.0 / D, None, OP.mult, OP.bypass
    )
    musq = statp.tile([128, 2, 394], F32, tag="st", name="musq")
    nc.vector.tensor_tensor(musq[:], mu[:], mu[:], OP.mult)
    var = statp.tile([128, 2, 394], F32, tag="st", name="var")
    nc.vector.scalar_tensor_tensor(
        var[:], sq_ps[:, :, 0:394], 1.0 / D, musq[:], OP.mult, OP.subtract
    )
    # rstd = exp(-0.5 * ln(var + eps))
    lnv = statp.tile([128, 2, 394], F32, tag="st", name="lnv")
    nc.scalar.activation(lnv[:], var[:], AF.Ln, bias=P["eps_sb"][:])
    rstd = statp.tile([128, 2, 394], F32, tag="st", name="rstd")
    nc.scalar.activation(rstd[:], lnv[:], AF.Exp, scale=-0.5)
    mr = statp.tile([128, 2, 394], F32, tag="st", name="mr")
    nc.vector.tensor_tensor(mr[:], mu[:], rstd[:], OP.mult)
    rstd_f = rstd[:].rearrange("p a b -> p (a b)")
    mr_f = mr[:].rearrange("p a b -> p (a b)")
    for ct in range(DC):
        nc.vector.tensor_tensor(z[:, ct, :], zf[:, ct, :], rstd_f[:, :788], OP.mult)
        nc.vector.tensor_tensor(z[:, ct, :], zf[:, ct, :], mr_f[:, :788], OP.subtract)
        nc.scalar.activation(
            x_out[:, ct, :],
            zf[:, ct, :],
            AF.Identity,
            bias=be_sb[:, ct : ct + 1],
            scale=g_sb[:, ct : ct + 1],
        )


_NC_CACHE = None


def _host_prep(inputs):
    """Patchify vid, build per-core inputs, pre-transpose weights (host-side)."""
    bf = ml_dtypes.bfloat16
    vid = np.asarray(inputs["vid"], np.float32)
    x = vid.reshape(B, L, C, H // PH, PH, W // PW, PW)
    x = x.transpose(0, 1, 3, 5, 4, 6, 2).reshape(B, L, NP, PD)

    pos = np.asarray(inputs["pos_emb"], np.float32)[0]  # [L, NP+1, D]
    cls = np.asarray(inputs["cls"], np.float32)[0, :, 0, :]  # [L, D]
    b_emb = np.asarray(inputs["b_embed"], np.float32)  # [D]

    shared = {
        "wembT": np.ascontiguousarray(
            np.asarray(inputs["W_embed"], np.float32).T
        ).astype(bf),
        "wqT": np.ascontiguousarray(
            np.asarray(inputs["Wq"], np.float32).transpose(0, 2, 1)
        ).astype(bf),
        "wkT": np.ascontiguousarray(
            np.asarray(inputs["Wk"], np.float32).transpose(0, 2, 1)
        ).astype(bf),
        "wvT": np.ascontiguousarray(
            np.asarray(inputs["Wv"], np.float32).transpose(0, 2, 1)
        ).astype(bf),
        "woT": np.ascontiguousarray(
            np.asarray(inputs["Wo"], np.float32).transpose(0, 2, 1)
        ).astype(bf),
        "w1T": np.ascontiguousarray(
            np.asarray(inputs["W1"], np.float32).transpose(0, 2, 1)
        ).astype(bf),
        "w2T": np.ascontiguousarray(
            np.asarray(inputs["W2"], np.float32).transpose(0, 2, 1)
        ).astype(bf),
        "bq": np.asarray(inputs["bq"], np.float32),
        "bk": np.asarray(inputs["bk"], np.float32),
        "bv": np.asarray(inputs["bv"], np.float32),
        "bo": np.asarray(inputs["bo"], np.float32),
        "b1": np.asarray(inputs["b1"], np.float32),
        "b2": np.asarray(inputs["b2"], np.float32),
        "g1": np.asarray(inputs["ln1_g"], np.float32),
        "be1": np.asarray(inputs["ln1_b"], np.float32),
        "g2": np.asarray(inputs["ln2_g"], np.float32),
        "be2": np.asarray(inputs["ln2_b"], np.float32),
        "ident": np.eye(128, dtype=np.float32).astype(bf),
    }

    in_maps = []
    for c in range(N_CORES):
        b, half = c // 2, c % 2
        f0 = half * (L // 2)
        pat_c = np.zeros((PD, OWN), np.float32)
        addv_c = np.zeros((D, OWN), np.float32)
        for f in range(L // 2):
            fr = f0 + f
            t0 = f * (NP + 1)
            pat_c[:, t0 + 1 : t0 + NP + 1] = x[b, fr].T
            addv_c[:, t0] = pos[fr, 0] + cls[fr]
            addv_c[:, t0 + 1 : t0 + NP + 1] = (
                pos[fr, 1:].T + b_emb[:, None]
            )
        m = {"pat": pat_c.astype(bf), "addv": addv_c}
        m.update(shared)
        in_maps.append(m)
    return in_maps


def kernel(**inputs):
    global _NC_CACHE
    in_maps = _host_prep(inputs)
    if _NC_CACHE is None:
        nc = build_kernel()
        n = legalize_waits(nc)
        _NC_CACHE = nc
    nc = _NC_CACHE
    res = run_bass_kernel_spmd(nc, in_maps, core_ids=list(range(N_CORES)))
    out = np.zeros((B, S, D), np.float32)
    for c in range(N_CORES):
        b, half = c // 2, c % 2
        out[b, half * OWN : (half + 1) * OWN, :] = res.results[c]["xout"]
    return out


# revision 16
# speedup vs baseline: 1.2472x; 1.2472x over previous
"""Trainium2 Bass kernel for nn_Encoder_37340445671714 (video ViT encoder).

Sharding: 8 cores = 4 batch elements x 2 sequence halves (788 tokens each).
Each core runs the full 6-layer encoder for its (batch, half):
  - activations kept transposed [feature, token] in SBUF
  - all matmuls bf16 (fp32 PSUM accumulate), residual stream bf16
  - flash-style attention: scores^T per k-tile -> Exp on ScalarE -> AV
    accumulation with 64 ones-columns in V replicating the softmax
    denominator Z across PSUM partitions 64:128 (normalization via
    exp(-ln Z) on ScalarE)
  - LayerNorm stats via ones-matmul partition sums on TensorE
  - per-layer pair AllGather (bf16) exchanges the updated half sequence
Weights are pre-transposed/padded on the host (free) into matmul-ready
layouts. Output is transposed back to natural layout on the PE at the end.
"""

import numpy as np
import ml_dtypes

import concourse.bass as bass
import concourse.tile as tile
from concourse import mybir
from concourse.bass_utils import run_bass_kernel_spmd

F32 = mybir.dt.float32
F32R = mybir.dt.float32r
BF16 = mybir.dt.bfloat16
AF = mybir.ActivationFunctionType
OP = mybir.AluOpType

# problem dims
B, L, C, H, W = 4, 8, 3, 224, 224
PH = PW = 16
D = 512
NH = 8
DK = 64
FF = 2048
NL = 6
NP = (H // PH) * (W // PW)  # 196
S = L * (NP + 1)  # 1576
PD = PH * PW * C  # 768
OWN = S // 2  # 788 tokens per core
LN_EPS = 1e-5

DC = D // 128  # 4 feature subtiles
PDC = PD // 128  # 6
FTC = FF // 128  # 16

# q chunks (the 2 halves of the own-token range)
QC = [(0, 394), (394, 394)]
# k tiles over the full sequence
KT = [(i * 128, 128) for i in range(S // 128)] + [(S - S % 128, S % 128)]  # 12x128+40

N_CORES = 8
REPLICA_GROUPS = [[0, 1], [2, 3], [4, 5], [6, 7]]


def legalize_waits(nc):
    """Split multi-wait instructions into preceding single-wait NoOps.

    The walrus build in this environment rejects instructions carrying more
    than one semaphore wait command.
    """
    n_split = 0
    for f in nc.m.functions:
        for bb in f.blocks:
            insts = list(bb.instructions)
            new_insts = []
            changed = False
            for inst in insts:
                si = inst.sync_info
                if si is not None and len(si.on_wait) > 1:
                    waits = list(si.on_wait)
                    for w in waits[:-1]:
                        nop = mybir.InstNoOp(
                            name=nc.get_next_instruction_name(),
                            engine=inst.engine,
                            ins=[],
                            outs=[],
                        )
                        nop.sync_info = mybir.SyncInfo(on_wait=[w], on_update=[])
                        new_insts.append(nop)
                        n_split += 1
                    inst.sync_info = mybir.SyncInfo(
                        on_wait=[waits[-1]], on_update=list(si.on_update)
                    )
                    changed = True
                new_insts.append(inst)
            if changed:
                bb.instructions = new_insts
    return n_split


def _bcast_ap(ap_1d, parts=128):
    """Partition-broadcast DRAM AP: [n] -> [parts, n] with partition stride 0."""
    return bass.AP(
        tensor=ap_1d.tensor, offset=ap_1d.offset, ap=[[0, parts]] + list(ap_1d.ap)
    )


def build_kernel():
    nc = bass.Bass(
        "TRN2", target_bir_lowering=False, debug=False, num_devices=N_CORES
    )

    # ---- I/O ----
    pat = nc.dram_tensor("pat", [PD, OWN], BF16, kind="ExternalInput").ap()
    addv = nc.dram_tensor("addv", [D, OWN], F32, kind="ExternalInput").ap()
    wembT = nc.dram_tensor("wembT", [PD, D], BF16, kind="ExternalInput").ap()
    wqT = nc.dram_tensor("wqT", [NL, D, D], BF16, kind="ExternalInput").ap()
    wkT = nc.dram_tensor("wkT", [NL, D, D], BF16, kind="ExternalInput").ap()
    wvT = nc.dram_tensor("wvxT", [NL, D, NH * 128], BF16, kind="ExternalInput").ap()
    woT = nc.dram_tensor("woT", [NL, D, D], BF16, kind="ExternalInput").ap()
    w1T = nc.dram_tensor("w1T", [NL, D, FF], BF16, kind="ExternalInput").ap()
    w2T = nc.dram_tensor("w2T", [NL, FF, D], BF16, kind="ExternalInput").ap()
    bq = nc.dram_tensor("bq", [NL, D], F32, kind="ExternalInput").ap()
    bk = nc.dram_tensor("bk", [NL, D], F32, kind="ExternalInput").ap()
    bv = nc.dram_tensor("bvx", [NL, NH * 128], F32, kind="ExternalInput").ap()
    bo = nc.dram_tensor("bo", [NL, D], F32, kind="ExternalInput").ap()
    b1 = nc.dram_tensor("b1", [NL, FF], F32, kind="ExternalInput").ap()
    b2 = nc.dram_tensor("b2", [NL, D], F32, kind="ExternalInput").ap()
    g1 = nc.dram_tensor("g1", [NL, D], F32, kind="ExternalInput").ap()
    be1 = nc.dram_tensor("be1", [NL, D], F32, kind="ExternalInput").ap()
    g2 = nc.dram_tensor("g2", [NL, D], F32, kind="ExternalInput").ap()
    be2 = nc.dram_tensor("be2", [NL, D], F32, kind="ExternalInput").ap()
    ident = nc.dram_tensor("ident", [128, 128], BF16, kind="ExternalInput").ap()
    swapid = nc.dram_tensor("swapid", [128, 128], F32R, kind="ExternalInput").ap()
    xout = nc.dram_tensor("xout", [OWN, D], F32, kind="ExternalOutput").ap()

    with tile.TileContext(nc) as tc:
        with (
            tc.tile_pool(name="const", bufs=1) as constp,
            tc.tile_pool(name="wsmall", bufs=1) as wsmall,
            tc.tile_pool(name="wff", bufs=3) as wff,
            tc.tile_pool(name="xp", bufs=2) as xp,
            tc.tile_pool(name="kv", bufs=2) as kvp,
            tc.tile_pool(name="vp", bufs=1) as vp,
            tc.tile_pool(name="qo", bufs=1) as qop,
            tc.tile_pool(name="zp", bufs=1) as zp,
            tc.tile_pool(name="zbp", bufs=1) as zbp,
            tc.tile_pool(name="big", bufs=1) as bigp,
            tc.tile_pool(name="exps", bufs=3) as expp,
            tc.tile_pool(name="stat", bufs=5) as statp,
            tc.tile_pool(name="rz", bufs=3) as rzp,
            tc.tile_pool(name="bias", bufs=2) as biasp,
            tc.tile_pool(name="psA", bufs=1, space="PSUM") as psA,
            tc.tile_pool(name="psB", bufs=2, space="PSUM") as psB,
            tc.tile_pool(name="dram", bufs=2, space="DRAM") as dramp,
        ):
            P = dict(
                constp=constp, wsmall=wsmall, wff=wff, xp=xp, kvp=kvp, vp=vp,
                qop=qop, zp=zp, zbp=zbp, bigp=bigp, expp=expp, statp=statp,
                rzp=rzp, biasp=biasp, psA=psA, psB=psB, dramp=dramp,
            )
            dram_in = dict(
                pat=pat, addv=addv, wembT=wembT, wqT=wqT, wkT=wkT, wvT=wvT,
                woT=woT, w1T=w1T, w2T=w2T, bq=bq, bk=bk, bv=bv, bo=bo, b1=b1, swapid=swapid,
                b2=b2, g1=g1, be1=be1, g2=g2, be2=be2, ident=ident, xout=xout,
            )
            ones_bf = constp.tile([128, 128], BF16, name="ones_bf")
            nc.vector.memset(ones_bf[:], 1.0)
            ones_r = constp.tile([128, 128], F32R, name="ones_r")
            nc.vector.tensor_scalar_add(ones_r[:], ones_bf[:], 0.0)
            ident_sb = constp.tile([128, 128], BF16, name="ident_sb")
            nc.sync.dma_start(ident_sb[:], ident[:])
            eps_sb = constp.tile([128, 1], F32, name="eps_sb")
            nc.vector.memset(eps_sb[:], LN_EPS)
            swap_sb = constp.tile([128, 128], F32R, name="swap_sb")
            nc.sync.dma_start(swap_sb[:], swapid[:])
            P["ones_bf"] = ones_bf
            P["ones_r"] = ones_r
            P["ident_sb"] = ident_sb
            P["eps_sb"] = eps_sb
            P["swap_sb"] = swap_sb

            x_bf = _embed(nc, P, dram_in)
            for l in range(NL):
                x_bf = _one_layer(nc, P, dram_in, l, x_bf)
            _tail(nc, P, dram_in, x_bf)
    return nc


def _embed(nc, P, dr):
    bigp, zp, wff, xp, psB = P["bigp"], P["zp"], P["wff"], P["xp"], P["psB"]
    pat_sb = bigp.tile([128, FTC, OWN], BF16, tag="h", name="pat_sb")
    nc.sync.dma_start(
        pat_sb[:, :PDC, :], dr["pat"].rearrange("(ko p) t -> p ko t", p=128)
    )
    addv_sb = zp.tile([128, DC, OWN], F32, tag="z", name="addv_sb")
    nc.sync.dma_start(addv_sb[:], dr["addv"].rearrange("(co p) t -> p co t", p=128))
    wemb_sb = wff.tile([128, PDC, D], BF16, tag="wff", name="wemb_sb")
    nc.sync.dma_start(wemb_sb[:], dr["wembT"].rearrange("(ko p) d -> p ko d", p=128))

    x_bf = xp.tile([128, DC, OWN], BF16, tag="x", name="x_emb")
    for dt in range(DC):
        for (q0, qn) in QC:
            ps = psB.tile([128, 2, 512], F32, tag="s", name="ps_emb")
            for kt in range(PDC):
                nc.tensor.matmul(
                    ps[:, 0, :qn],
                    wemb_sb[:, kt, dt * 128 : (dt + 1) * 128],
                    pat_sb[:, kt, q0 : q0 + qn],
                    start=(kt == 0),
                    stop=(kt == PDC - 1),
                )
            nc.vector.tensor_tensor(
                x_bf[:, dt, q0 : q0 + qn],
                ps[:, 0, :qn],
                addv_sb[:, dt, q0 : q0 + qn],
                OP.add,
            )
    return x_bf


def _load_layer_params(nc, P, dr, l):
    biasp, wsmall = P["biasp"], P["wsmall"]
    prm = {}
    for nm in ["bq", "bk", "bo", "b2", "g1", "be1", "g2", "be2"]:
        t = biasp.tile([128, DC], F32, tag=nm, name=nm + "_sb")
        nc.sync.dma_start(t[:], dr[nm][l].rearrange("(o p) -> p o", p=128))
        prm[nm] = t
    b1_sb = biasp.tile([128, FTC], F32, tag="b1", name="b1_sb")
    nc.sync.dma_start(b1_sb[:], dr["b1"][l].rearrange("(o p) -> p o", p=128))
    prm["b1"] = b1_sb
    bv_bc = biasp.tile([128, NH * 128], BF16, tag="bvb", name="bv_bc")
    nc.gpsimd.dma_start(bv_bc[:], _bcast_ap(dr["bv"][l]))
    prm["bv_bc"] = bv_bc
    for nm, key in [("wq", "wqT"), ("wk", "wkT"), ("wo", "woT")]:
        t = wsmall.tile([128, DC, D], BF16, tag=nm, name=nm + "_sb")
        nc.sync.dma_start(t[:], dr[key][l].rearrange("(co p) d -> p co d", p=128))
        prm[nm] = t
    wv = wsmall.tile([128, DC, NH * 128], BF16, tag="wv", name="wv_sb")
    nc.sync.dma_start(wv[:], dr["wvT"][l].rearrange("(co p) d -> p co d", p=128))
    prm["wv"] = wv
    return prm


def _allgather_x(nc, P, x_bf):
    dramp, kvp = P["dramp"], P["kvp"]
    xg_in = dramp.tile([DC, 128, OWN], BF16, tag="agi", name="xg_in")
    nc.sync.dma_start(xg_in[:].rearrange("c p t -> p c t"), x_bf[:])
    xg_out = dramp.tile([2, DC, 128, OWN], BF16, tag="ago", name="xg_out")
    nc.gpsimd.collective_compute(
        "AllGather",
        OP.bypass,
        replica_groups=REPLICA_GROUPS,
        ins=[xg_in[:].opt()],
        outs=[xg_out[:].opt()],
    )
    x_all = kvp.tile([128, DC, S], BF16, tag="kv", name="x_all")
    for s_ in range(2):
        nc.sync.dma_start(
            x_all[:, :, s_ * OWN : (s_ + 1) * OWN],
            xg_out[s_].rearrange("c p t -> p c t"),
        )
    return x_all


def _proj_to_T(nc, psB, w_sb, rhs, out_sb, bias_sb, chunks, n_ct=DC):
    """out_sb[:, dt, chunk] (bf16, transposed) = w^T-style proj + per-partition bias."""
    for dt in range(DC):
        for (t0, tn) in chunks:
            ps = psB.tile([128, 2, 512], F32, tag="s", name="ps_p")
            for ct in range(n_ct):
                nc.tensor.matmul(
                    ps[:, 0, :tn],
                    w_sb[:, ct, dt * 128 : (dt + 1) * 128],
                    rhs[:, ct, t0 : t0 + tn],
                    start=(ct == 0),
                    stop=(ct == n_ct - 1),
                )
            nc.vector.tensor_scalar_add(
                out_sb[:, dt, t0 : t0 + tn], ps[:, 0, :tn], bias_sb[:, dt : dt + 1]
            )


def _attention(nc, P, prm, x_bf, x_all):
    psA, psB, qop, vp, expp, rzp = (
        P["psA"], P["psB"], P["qop"], P["vp"], P["expp"], P["rzp"]
    )
    ones_bf = P["ones_bf"]
    qT = qop.tile([128, DC, OWN], BF16, tag="qT", name="qT")
    _proj_to_T(nc, psB, prm["wq"], x_bf, qT, prm["bq"], QC)
    kT = P["kvp"].tile([128, DC, S], BF16, tag="kv", name="kT")
    _proj_to_T(
        nc, psB, prm["wk"], x_all, kT, prm["bk"], [(i * 394, 394) for i in range(4)]
    )

    v_bf = vp.tile([128, len(KT), NH, 128], BF16, tag="v", name="v_bf")
    for ti, (k0, ksz) in enumerate(KT):
        ps = psB.tile([128, 2, 512], F32, tag="s", name="ps_v")
        for half in range(2):
            for ct in range(DC):
                nc.tensor.matmul(
                    ps[:ksz, half, :512],
                    x_all[:, ct, k0 : k0 + ksz],
                    prm["wv"][:, ct, half * 512 : (half + 1) * 512],
                    start=(ct == 0),
                    stop=(ct == DC - 1),
                )
        nc.vector.tensor_tensor(
            v_bf[:ksz, ti, :, :],
            ps[:ksz, :, :].rearrange("t a b -> t (a b)").rearrange(
                "t (h e) -> t h e", h=NH
            ),
            prm["bv_bc"][:ksz, :].rearrange("t (h e) -> t h e", h=NH),
            OP.add,
        )

    oT = qop.tile([128, DC, OWN], BF16, tag="oT", name="oT")
    for hpair in range(NH // 2):
        hdt = hpair
        # P1: head even -> rows 0:64 = o_e, 64:128 = Z_e (ones-half of V')
        # P2: head odd  -> rows 0:64 = Z_o, 64:128 = o_o
        p1 = psA.tile([128, 2, 512], F32, tag="o", name="p1")
        p2 = psA.tile([128, 2, 512], F32, tag="zz", name="p2")
        for ti, (k0, ksz) in enumerate(KT):
            first, last = (ti == 0), (ti == len(KT) - 1)
            for sub in range(2):
                hp = sub * 64
                s_ps = psB.tile([128, 2, 512], F32, tag="s", name="s_ps")
                for ci, (q0, qn) in enumerate(QC):
                    nc.tensor.matmul(
                        s_ps[:ksz, ci, :qn],
                        kT[hp : hp + 64, hdt, k0 : k0 + ksz],
                        qT[hp : hp + 64, hdt, q0 : q0 + qn],
                        start=True,
                        stop=True,
                    )
                e_sb = expp.tile([128, 2, 394], BF16, tag="e", name="expS")
                nc.scalar.activation(
                    e_sb[:ksz, :, :], s_ps[:ksz, :, 0:394], AF.Exp, scale=0.125
                )
                dst = p1 if sub == 0 else p2
                for ci, (q0, qn) in enumerate(QC):
                    nc.tensor.matmul(
                        dst[:, ci, :qn],
                        v_bf[:ksz, ti, 2 * hpair + sub, :],
                        e_sb[:ksz, ci, :qn],
                        start=first,
                        stop=last,
                    )
        # 1/Z: Z_o in p2[0:64], Z_e in p1[64:128]; ln+exp base-aligned,
        # then swap halves with a f32r anti-diagonal-identity matmul.
        lnmix = rzp.tile([128, 2, 394], F32, tag="rz", name="lnmix")
        nc.scalar.activation(lnmix[0:64, :, :], p2[0:64, :, 0:394], AF.Ln)
        nc.scalar.activation(lnmix[64:128, :, :], p1[64:128, :, 0:394], AF.Ln)
        rzmix = rzp.tile([128, 2, 394], F32R, tag="rz", name="rzmix")
        nc.scalar.activation(rzmix[:], lnmix[:], AF.Exp, scale=-1.0)
        rsw_ps = psB.tile([128, 2, 512], F32, tag="s", name="rsw_ps")
        for ci in range(2):
            nc.tensor.matmul(
                rsw_ps[:, ci, :394],
                P["swap_sb"][:],
                rzmix[:, ci, :],
                start=True,
                stop=True,
            )
        rzs = rzp.tile([128, 2, 394], F32, tag="rz", name="rzs")
        nc.scalar.activation(rzs[:], rsw_ps[:, :, 0:394], AF.Identity)
        nc.vector.tensor_tensor(
            oT[0:64, hdt, :].rearrange("p (a b) -> p a b", a=2),
            p1[0:64, :, 0:394],
            rzs[0:64, :, :],
            OP.mult,
        )
        nc.vector.tensor_tensor(
            oT[64:128, hdt, :].rearrange("p (a b) -> p a b", a=2),
            p2[64:128, :, 0:394],
            rzs[64:128, :, :],
            OP.mult,
        )
    return oT


def _residual_proj(nc, psB, w_sb, rhs_T, bias_sb, x_bf, z_out):
    """z_out (f32) = W^T proj of rhs_T + bias + x_bf (residual)."""
    for dt in range(DC):
        for (q0, qn) in QC:
            ps = psB.tile([128, 2, 512], F32, tag="s", name="ps_r")
            for ct in range(DC):
                nc.tensor.matmul(
                    ps[:, 0, :qn],
                    w_sb[:, ct, dt * 128 : (dt + 1) * 128],
                    rhs_T[:, ct, q0 : q0 + qn],
                    start=(ct == 0),
                    stop=(ct == DC - 1),
                )
            nc.vector.scalar_tensor_tensor(
                z_out[:, dt, q0 : q0 + qn],
                ps[:, 0, :qn],
                bias_sb[:, dt : dt + 1],
                x_bf[:, dt, q0 : q0 + qn],
                OP.add,
                OP.add,
            )


def _ffn(nc, P, dr, l, prm, x_bf, z_out):
    psA, psB, bigp, wff = P["psA"], P["psB"], P["bigp"], P["wff"]
    h_bf = bigp.tile([128, FTC, OWN], BF16, tag="h", name="h_bf")
    for half in range(2):
        w1_sb = wff.tile([128, DC, FF // 2], BF16, tag="wff", name="w1_sb")
        nc.sync.dma_start(
            w1_sb[:],
            dr["w1T"][l][:, half * (FF // 2) : (half + 1) * (FF // 2)].rearrange(
                "(co p) f -> p co f", p=128
            ),
        )
        for j in range(FTC // 2):
            ft = half * (FTC // 2) + j
            ps = psB.tile([128, 2, 512], F32, tag="s", name="ps_h")
            for ci, (q0, qn) in enumerate(QC):
                for ct in range(DC):
                    nc.tensor.matmul(
                        ps[:, ci, :qn],
                        w1_sb[:, ct, j * 128 : (j + 1) * 128],
                        x_bf[:, ct, q0 : q0 + qn],
                        start=(ct == 0),
                        stop=(ct == DC - 1),
                    )
            nc.vector.tensor_scalar(
                h_bf[:, ft, :].rearrange("p (a b) -> p a b", a=2),
                ps[:, :, 0:394],
                prm["b1"][:, ft : ft + 1],
                0.0,
                OP.add,
                OP.max,
            )

    w2_halves = []
    for half in range(2):
        w2_sb = wff.tile([128, FTC // 2, D], BF16, tag="wff", name="w2_sb")
        nc.sync.dma_start(
            w2_sb[:],
            dr["w2T"][l][half * (FF // 2) : (half + 1) * (FF // 2), :].rearrange(
                "(fo p) d -> p fo d", p=128
            ),
        )
        w2_halves.append(w2_sb)
    for dt in range(DC):
        for (q0, qn) in QC:
            ps2 = psA.tile([128, 2, 512], F32, tag="o", name="ps_f")
            for half in range(2):
                for j in range(FTC // 2):
                    ft = half * (FTC // 2) + j
                    nc.tensor.matmul(
                        ps2[:, 0, :qn],
                        w2_halves[half][:, j, dt * 128 : (dt + 1) * 128],
                        h_bf[:, ft, q0 : q0 + qn],
                        start=(ft == 0),
                        stop=(ft == FTC - 1),
                    )
            nc.vector.scalar_tensor_tensor(
                z_out[:, dt, q0 : q0 + qn],
                ps2[:, 0, :qn],
                prm["b2"][:, dt : dt + 1],
                x_bf[:, dt, q0 : q0 + qn],
                OP.add,
                OP.add,
            )


def _one_layer(nc, P, dr, l, x_bf):
    prm = _load_layer_params(nc, P, dr, l)
    x_all = _allgather_x(nc, P, x_bf)
    oT = _attention(nc, P, prm, x_bf, x_all)
    z = P["zp"].tile([128, DC, OWN], F32R, tag="z", name=f"z1_{l}")
    _residual_proj(nc, P["psB"], prm["wo"], oT, prm["bo"], x_bf, z)
    x_bf = P["xp"].tile([128, DC, OWN], BF16, tag="x", name=f"x_ln1_{l}")
    _layernorm(nc, P, z, x_bf, prm["g1"], prm["be1"])
    z = P["zp"].tile([128, DC, OWN], F32R, tag="z", name=f"z2_{l}")
    _ffn(nc, P, dr, l, prm, x_bf, z)
    last = l == NL - 1
    if last:
        x2 = P["bigp"].tile([128, DC, OWN], F32, tag="h", name="x_final")
    else:
        x2 = P["xp"].tile([128, DC, OWN], BF16, tag="x", name=f"x_ln2_{l}")
    _layernorm(nc, P, z, x2, prm["g2"], prm["be2"])
    return x2


def _tail(nc, P, dr, x_f32):
    psB, bigp = P["psB"], P["bigp"]
    xout = dr["xout"]
    ident32 = P["constp"].tile([128, 128], F32, name="ident32")
    nc.vector.tensor_scalar_add(ident32[:], P["ident_sb"][:], 0.0)
    for ti in range(7):
        t0 = ti * 128
        tsz = min(128, OWN - t0)
        xo_sb = P["statp"].tile([128, D], F32, tag="st", name="xo_sb")
        for dt in range(DC):
            tp = psB.tile([128, 2, 512], F32, tag="s", name="tp")
            nc.tensor.transpose(
                tp[:tsz, 0, :128], x_f32[:, dt, t0 : t0 + tsz], ident32[:]
            )
            nc.vector.tensor_scalar_add(
                xo_sb[:tsz, dt * 128 : (dt + 1) * 128], tp[:tsz, 0, :128], 0.0
            )
        nc.sync.dma_start(xout[t0 : t0 + tsz, :], xo_sb[:tsz, :])


def _layernorm(nc, P, z, x_out, g_sb, be_sb):
    """Post-LN over features (partition dim) in transposed layout.

    z: [128, DC, OWN] f32r.  Writes x_out = (z - mu) * rstd * g + b.
    Stats are computed with f32r ones-matmuls directly on z (full precision
    vs the bf16 shadow it replaces)."""
    psB, statp, zbp = P["psB"], P["statp"], P["zbp"]
    ones_r = P["ones_r"]
    zf = z[:].bitcast(F32)
    sum_ps = psB.tile([128, 2, 512], F32, tag="s", name="sum_ps")
    for ci, (q0, qn) in enumerate(QC):
        for ct in range(DC):
            nc.tensor.matmul(
                sum_ps[:, ci, :qn],
                ones_r[:],
                z[:, ct, q0 : q0 + qn],
                start=(ct == 0),
                stop=(ct == DC - 1),
            )
    sq = zbp.tile([128, DC, OWN], F32R, tag="zb", name="sq_r")
    nc.vector.tensor_tensor(sq[:], zf, zf, OP.mult)
    sq_ps = psB.tile([128, 2, 512], F32, tag="s", name="sq_ps")
    for ci, (q0, qn) in enumerate(QC):
        for ct in range(DC):
            nc.tensor.matmul(
                sq_ps[:, ci, :qn],
                ones_r[:],
                sq[:, ct, q0 : q0 + qn],
                start=(ct == 0),
                stop=(ct == DC - 1),
            )
    mu = statp.tile([128, 2, 394], F32, tag="st", name="mu")
    nc.vector.tensor_scalar(
        mu[:], sum_ps[:, :, 0:394], 1.0 / D, None, OP.mult, OP.bypass
    )
    musq = statp.tile([128, 2, 394], F32, tag="st", name="musq")
    nc.vector.tensor_tensor(musq[:], mu[:], mu[:], OP.mult)
    var = statp.tile([128, 2, 394], F32, tag="st", name="var")
    nc.vector.scalar_tensor_tensor(
        var[:], sq_ps[:, :, 0:394], 1.0 / D, musq[:], OP.mult, OP.subtract
    )
    # rstd = exp(-0.5 * ln(var + eps))
    lnv = statp.tile([128, 2, 394], F32, tag="st", name="lnv")
    nc.scalar.activation(lnv[:], var[:], AF.Ln, bias=P["eps_sb"][:])
    rstd = statp.tile([128, 2, 394], F32, tag="st", name="rstd")
    nc.scalar.activation(rstd[:], lnv[:], AF.Exp, scale=-0.5)
    mr = statp.tile([128, 2, 394], F32, tag="st", name="mr")
    nc.vector.tensor_tensor(mr[:], mu[:], rstd[:], OP.mult)
    rstd_f = rstd[:].rearrange("p a b -> p (a b)")
    mr_f = mr[:].rearrange("p a b -> p (a b)")
    for ct in range(DC):
        nc.vector.tensor_tensor(z[:, ct, :], zf[:, ct, :], rstd_f[:, :788], OP.mult)
        nc.vector.tensor_tensor(z[:, ct, :], zf[:, ct, :], mr_f[:, :788], OP.subtract)
        nc.scalar.activation(
            x_out[:, ct, :],
            zf[:, ct, :],
            AF.Identity,
            bias=be_sb[:, ct : ct + 1],
            scale=g_sb[:, ct : ct + 1],
        )


def _build_wvx(Wv):
    """Extend Wv^T to [NL, D, NH*128]: per head a 64-col V block and a 64-col
    zero block (ones come from the bias); even heads [V|0], odd heads [0|V]."""
    bf = ml_dtypes.bfloat16
    WvT = Wv.transpose(0, 2, 1)  # [NL, D(c), D(v)]
    out = np.zeros((NL, D, NH * 128), np.float32)
    for h in range(NH):
        off = h * 128 + (0 if h % 2 == 0 else 64)
        out[:, :, off : off + 64] = WvT[:, :, h * 64 : (h + 1) * 64]
    return out.astype(bf)


def _build_bvx(bv):
    """Bias for the extended V: per head the V-half gets bv, ones-half gets 1."""
    out = np.ones((NL, NH * 128), np.float32)
    for h in range(NH):
        off = h * 128 + (0 if h % 2 == 0 else 64)
        out[:, off : off + 64] = bv[:, h * 64 : (h + 1) * 64]
    return out


_NC_CACHE = None


def _host_prep(inputs):
    """Patchify vid, build per-core inputs, pre-transpose weights (host-side)."""
    bf = ml_dtypes.bfloat16
    vid = np.asarray(inputs["vid"], np.float32)
    x = vid.reshape(B, L, C, H // PH, PH, W // PW, PW)
    x = x.transpose(0, 1, 3, 5, 4, 6, 2).reshape(B, L, NP, PD)

    pos = np.asarray(inputs["pos_emb"], np.float32)[0]  # [L, NP+1, D]
    cls = np.asarray(inputs["cls"], np.float32)[0, :, 0, :]  # [L, D]
    b_emb = np.asarray(inputs["b_embed"], np.float32)  # [D]

    shared = {
        "wembT": np.ascontiguousarray(
            np.asarray(inputs["W_embed"], np.float32).T
        ).astype(bf),
        "wqT": np.ascontiguousarray(
            np.asarray(inputs["Wq"], np.float32).transpose(0, 2, 1)
        ).astype(bf),
        "wkT": np.ascontiguousarray(
            np.asarray(inputs["Wk"], np.float32).transpose(0, 2, 1)
        ).astype(bf),
        "wvxT": _build_wvx(np.asarray(inputs["Wv"], np.float32)),
        "woT": np.ascontiguousarray(
            np.asarray(inputs["Wo"], np.float32).transpose(0, 2, 1)
        ).astype(bf),
        "w1T": np.ascontiguousarray(
            np.asarray(inputs["W1"], np.float32).transpose(0, 2, 1)
        ).astype(bf),
        "w2T": np.ascontiguousarray(
            np.asarray(inputs["W2"], np.float32).transpose(0, 2, 1)
        ).astype(bf),
        "bq": np.asarray(inputs["bq"], np.float32),
        "bk": np.asarray(inputs["bk"], np.float32),
        "bvx": _build_bvx(np.asarray(inputs["bv"], np.float32)),
        "bo": np.asarray(inputs["bo"], np.float32),
        "b1": np.asarray(inputs["b1"], np.float32),
        "b2": np.asarray(inputs["b2"], np.float32),
        "g1": np.asarray(inputs["ln1_g"], np.float32),
        "be1": np.asarray(inputs["ln1_b"], np.float32),
        "g2": np.asarray(inputs["ln2_g"], np.float32),
        "be2": np.asarray(inputs["ln2_b"], np.float32),
        "ident": np.eye(128, dtype=np.float32).astype(bf),
        "swapid": np.roll(np.eye(128, dtype=np.float32), 64, axis=1),
    }

    in_maps = []
    for c in range(N_CORES):
        b, half = c // 2, c % 2
        f0 = half * (L // 2)
        pat_c = np.zeros((PD, OWN), np.float32)
        addv_c = np.zeros((D, OWN), np.float32)
        for f in range(L // 2):
            fr = f0 + f
            t0 = f * (NP + 1)
            pat_c[:, t0 + 1 : t0 + NP + 1] = x[b, fr].T
            addv_c[:, t0] = pos[fr, 0] + cls[fr]
            addv_c[:, t0 + 1 : t0 + NP + 1] = (
                pos[fr, 1:].T + b_emb[:, None]
            )
        m = {"pat": pat_c.astype(bf), "addv": addv_c}
        m.update(shared)
        in_maps.append(m)
    return in_maps


def kernel(**inputs):
    global _NC_CACHE
    in_maps = _host_prep(inputs)
    if _NC_CACHE is None:
        nc = build_kernel()
        n = legalize_waits(nc)
        _NC_CACHE = nc
    nc = _NC_CACHE
    res = run_bass_kernel_spmd(nc, in_maps, core_ids=list(range(N_CORES)))
    out = np.zeros((B, S, D), np.float32)
    for c in range(N_CORES):
        b, half = c // 2, c % 2
        out[b, half * OWN : (half + 1) * OWN, :] = res.results[c]["xout"]
    return out


# revision 18
# speedup vs baseline: 1.5462x; 1.2397x over previous
"""Trainium2 Bass kernel for nn_Encoder_37340445671714 (video ViT encoder).

Sharding: 8 cores = 4 batch elements x 2 sequence halves (788 tokens each).
Each core runs the full 6-layer encoder for its (batch, half):
  - activations kept transposed [feature, token] in SBUF
  - all matmuls bf16 (fp32 PSUM accumulate), residual stream bf16
  - flash-style attention: scores^T per k-tile -> Exp on ScalarE -> AV
    accumulation; V weights are host-extended per head with a 64-wide
    zero-weight/bias-1 block (parity-swapped for odd heads) so the same
    AV matmul also produces the softmax denominator Z; 1/Z = exp(-ln Z)
    on ScalarE, re-aligned to its head's partitions by one f32r matmul
    against a host anti-diagonal identity
  - LayerNorm stats via ones-matmul partition sums on TensorE
  - per-layer pair AllGather (bf16) exchanges the updated half sequence
Weights are pre-transposed/padded on the host (free) into matmul-ready
layouts. Output is transposed back to natural layout on the PE at the end.
"""

import numpy as np
import ml_dtypes

import concourse.bass as bass
import concourse.tile as tile
from concourse import mybir
from concourse.bass_utils import run_bass_kernel_spmd

F32 = mybir.dt.float32
F32R = mybir.dt.float32r
BF16 = mybir.dt.bfloat16
AF = mybir.ActivationFunctionType
OP = mybir.AluOpType

# problem dims
B, L, C, H, W = 4, 8, 3, 224, 224
PH = PW = 16
D = 512
NH = 8
DK = 64
FF = 2048
NL = 6
NP = (H // PH) * (W // PW)  # 196
S = L * (NP + 1)  # 1576
PD = PH * PW * C  # 768
OWN = S // 2  # 788 tokens per core
LN_EPS = 1e-5

DC = D // 128  # 4 feature subtiles
PDC = PD // 128  # 6
FTC = FF // 128  # 16

# q chunks (the 2 halves of the own-token range)
QC = [(0, 394), (394, 394)]
# k tiles over the full sequence
KT = [(i * 128, 128) for i in range(S // 128)] + [(S - S % 128, S % 128)]  # 12x128+40

N_CORES = 8
REPLICA_GROUPS = [[0, 1], [2, 3], [4, 5], [6, 7]]


def legalize_waits(nc):
    """Split multi-wait instructions into preceding single-wait NoOps.

    The walrus build in this environment rejects instructions carrying more
    than one semaphore wait command.
    """
    n_split = 0
    for f in nc.m.functions:
        for bb in f.blocks:
            insts = list(bb.instructions)
            new_insts = []
            changed = False
            for inst in insts:
                si = inst.sync_info
                if si is not None and len(si.on_wait) > 1:
                    waits = list(si.on_wait)
                    for w in waits[:-1]:
                        nop = mybir.InstNoOp(
                            name=nc.get_next_instruction_name(),
                            engine=inst.engine,
                            ins=[],
                            outs=[],
                        )
                        nop.sync_info = mybir.SyncInfo(on_wait=[w], on_update=[])
                        new_insts.append(nop)
                        n_split += 1
                    inst.sync_info = mybir.SyncInfo(
                        on_wait=[waits[-1]], on_update=list(si.on_update)
                    )
                    changed = True
                new_insts.append(inst)
            if changed:
                bb.instructions = new_insts
    return n_split


def _bcast_ap(ap_1d, parts=128):
    """Partition-broadcast DRAM AP: [n] -> [parts, n] with partition stride 0."""
    return bass.AP(
        tensor=ap_1d.tensor, offset=ap_1d.offset, ap=[[0, parts]] + list(ap_1d.ap)
    )


def build_kernel():
    nc = bass.Bass(
        "TRN2", target_bir_lowering=False, debug=False, num_devices=N_CORES
    )

    # ---- I/O ----
    pat = nc.dram_tensor("pat", [PD, OWN], BF16, kind="ExternalInput").ap()
    addv = nc.dram_tensor("addv", [D, OWN], F32, kind="ExternalInput").ap()
    wembT = nc.dram_tensor("wembT", [PD, D], BF16, kind="ExternalInput").ap()
    wqT = nc.dram_tensor("wqT", [NL, D, D], BF16, kind="ExternalInput").ap()
    wkT = nc.dram_tensor("wkT", [NL, D, D], BF16, kind="ExternalInput").ap()
    wvT = nc.dram_tensor("wvxT", [NL, D, NH * 128], BF16, kind="ExternalInput").ap()
    woT = nc.dram_tensor("woT", [NL, D, D], BF16, kind="ExternalInput").ap()
    w1T = nc.dram_tensor("w1T", [NL, D, FF], BF16, kind="ExternalInput").ap()
    w2T = nc.dram_tensor("w2T", [NL, FF, D], BF16, kind="ExternalInput").ap()
    bq = nc.dram_tensor("bq", [NL, D], F32, kind="ExternalInput").ap()
    bk = nc.dram_tensor("bk", [NL, D], F32, kind="ExternalInput").ap()
    bv = nc.dram_tensor("bvx", [NL, NH * 128], F32, kind="ExternalInput").ap()
    bo = nc.dram_tensor("bo", [NL, D], F32, kind="ExternalInput").ap()
    b1 = nc.dram_tensor("b1", [NL, FF], F32, kind="ExternalInput").ap()
    b2 = nc.dram_tensor("b2", [NL, D], F32, kind="ExternalInput").ap()
    g1 = nc.dram_tensor("g1", [NL, D], F32, kind="ExternalInput").ap()
    be1 = nc.dram_tensor("be1", [NL, D], F32, kind="ExternalInput").ap()
    g2 = nc.dram_tensor("g2", [NL, D], F32, kind="ExternalInput").ap()
    be2 = nc.dram_tensor("be2", [NL, D], F32, kind="ExternalInput").ap()
    ident = nc.dram_tensor("ident", [128, 128], BF16, kind="ExternalInput").ap()
    swapid = nc.dram_tensor("swapid", [128, 128], F32R, kind="ExternalInput").ap()
    xout = nc.dram_tensor("xout", [OWN, D], F32, kind="ExternalOutput").ap()

    with tile.TileContext(nc) as tc:
        with (
            tc.tile_pool(name="const", bufs=1) as constp,
            tc.tile_pool(name="wsmall", bufs=1) as wsmall,
            tc.tile_pool(name="wff", bufs=3) as wff,
            tc.tile_pool(name="xp", bufs=2) as xp,
            tc.tile_pool(name="kv", bufs=2) as kvp,
            tc.tile_pool(name="vp", bufs=1) as vp,
            tc.tile_pool(name="qo", bufs=1) as qop,
            tc.tile_pool(name="zp", bufs=1) as zp,
            tc.tile_pool(name="zbp", bufs=1) as zbp,
            tc.tile_pool(name="big", bufs=1) as bigp,
            tc.tile_pool(name="exps", bufs=3) as expp,
            tc.tile_pool(name="stat", bufs=5) as statp,
            tc.tile_pool(name="rz", bufs=3) as rzp,
            tc.tile_pool(name="bias", bufs=2) as biasp,
            tc.tile_pool(name="psA", bufs=1, space="PSUM") as psA,
            tc.tile_pool(name="psB", bufs=2, space="PSUM") as psB,
            tc.tile_pool(name="dram", bufs=2, space="DRAM") as dramp,
        ):
            P = dict(
                constp=constp, wsmall=wsmall, wff=wff, xp=xp, kvp=kvp, vp=vp,
                qop=qop, zp=zp, zbp=zbp, bigp=bigp, expp=expp, statp=statp,
                rzp=rzp, biasp=biasp, psA=psA, psB=psB, dramp=dramp,
            )
            dram_in = dict(
                pat=pat, addv=addv, wembT=wembT, wqT=wqT, wkT=wkT, wvT=wvT,
                woT=woT, w1T=w1T, w2T=w2T, bq=bq, bk=bk, bv=bv, bo=bo, b1=b1, swapid=swapid,
                b2=b2, g1=g1, be1=be1, g2=g2, be2=be2, ident=ident, xout=xout,
            )
            ones_bf = constp.tile([128, 128], BF16, name="ones_bf")
            nc.vector.memset(ones_bf[:], 1.0)
            ones_r = constp.tile([128, 128], F32R, name="ones_r")
            nc.vector.tensor_scalar_add(ones_r[:], ones_bf[:], 0.0)
            ident_sb = constp.tile([128, 128], BF16, name="ident_sb")
            nc.sync.dma_start(ident_sb[:], ident[:])
            eps_sb = constp.tile([128, 1], F32, name="eps_sb")
            nc.vector.memset(eps_sb[:], LN_EPS)
            swap_sb = constp.tile([128, 128], F32R, name="swap_sb")
            nc.sync.dma_start(swap_sb[:], swapid[:])
            P["ones_bf"] = ones_bf
            P["ones_r"] = ones_r
            P["ident_sb"] = ident_sb
            P["eps_sb"] = eps_sb
            P["swap_sb"] = swap_sb

            x_bf = _embed(nc, P, dram_in)
            for l in range(NL):
                x_bf = _one_layer(nc, P, dram_in, l, x_bf)
            _tail(nc, P, dram_in, x_bf)
    return nc


def _embed(nc, P, dr):
    bigp, zp, wff, xp, psB = P["bigp"], P["zp"], P["wff"], P["xp"], P["psB"]
    pat_sb = bigp.tile([128, FTC, OWN], BF16, tag="h", name="pat_sb")
    nc.sync.dma_start(
        pat_sb[:, :PDC, :], dr["pat"].rearrange("(ko p) t -> p ko t", p=128)
    )
    addv_sb = zp.tile([128, DC, OWN], F32, tag="z", name="addv_sb")
    nc.sync.dma_start(addv_sb[:], dr["addv"].rearrange("(co p) t -> p co t", p=128))
    wemb_sb = wff.tile([128, PDC, D], BF16, tag="wff", name="wemb_sb")
    nc.sync.dma_start(wemb_sb[:], dr["wembT"].rearrange("(ko p) d -> p ko d", p=128))

    x_bf = xp.tile([128, DC, OWN], BF16, tag="x", name="x_emb")
    for dt in range(DC):
        for (q0, qn) in QC:
            ps = psB.tile([128, 2, 512], F32, tag="s", name="ps_emb")
            for kt in range(PDC):
                nc.tensor.matmul(
                    ps[:, 0, :qn],
                    wemb_sb[:, kt, dt * 128 : (dt + 1) * 128],
                    pat_sb[:, kt, q0 : q0 + qn],
                    start=(kt == 0),
                    stop=(kt == PDC - 1),
                )
            nc.vector.tensor_tensor(
                x_bf[:, dt, q0 : q0 + qn],
                ps[:, 0, :qn],
                addv_sb[:, dt, q0 : q0 + qn],
                OP.add,
            )
    return x_bf


def _load_layer_params(nc, P, dr, l):
    biasp, wsmall = P["biasp"], P["wsmall"]
    prm = {}
    for nm in ["bq", "bk", "bo", "b2", "g1", "be1", "g2", "be2"]:
        t = biasp.tile([128, DC], F32, tag=nm, name=nm + "_sb")
        nc.sync.dma_start(t[:], dr[nm][l].rearrange("(o p) -> p o", p=128))
        prm[nm] = t
    b1_sb = biasp.tile([128, FTC], F32, tag="b1", name="b1_sb")
    nc.sync.dma_start(b1_sb[:], dr["b1"][l].rearrange("(o p) -> p o", p=128))
    prm["b1"] = b1_sb
    bv_bc = biasp.tile([128, NH * 128], BF16, tag="bvb", name="bv_bc")
    nc.gpsimd.dma_start(bv_bc[:], _bcast_ap(dr["bv"][l]))
    prm["bv_bc"] = bv_bc
    for nm, key in [("wq", "wqT"), ("wk", "wkT"), ("wo", "woT")]:
        t = wsmall.tile([128, DC, D], BF16, tag=nm, name=nm + "_sb")
        nc.sync.dma_start(t[:], dr[key][l].rearrange("(co p) d -> p co d", p=128))
        prm[nm] = t
    wv = wsmall.tile([128, DC, NH * 128], BF16, tag="wv", name="wv_sb")
    nc.sync.dma_start(wv[:], dr["wvT"][l].rearrange("(co p) d -> p co d", p=128))
    prm["wv"] = wv
    return prm


def _allgather_x(nc, P, x_bf):
    dramp, kvp = P["dramp"], P["kvp"]
    xg_in = dramp.tile([DC, 128, OWN], BF16, tag="agi", name="xg_in")
    nc.sync.dma_start(xg_in[:].rearrange("c p t -> p c t"), x_bf[:])
    xg_out = dramp.tile([2, DC, 128, OWN], BF16, tag="ago", name="xg_out")
    nc.gpsimd.collective_compute(
        "AllGather",
        OP.bypass,
        replica_groups=REPLICA_GROUPS,
        ins=[xg_in[:].opt()],
        outs=[xg_out[:].opt()],
    )
    x_all = kvp.tile([128, DC, S], BF16, tag="kv", name="x_all")
    for s_ in range(2):
        nc.sync.dma_start(
            x_all[:, :, s_ * OWN : (s_ + 1) * OWN],
            xg_out[s_].rearrange("c p t -> p c t"),
        )
    return x_all


def _proj_to_T(nc, psB, w_sb, rhs, out_sb, bias_sb, chunks, n_ct=DC):
    """out_sb[:, dt, chunk] (bf16, transposed) = w^T-style proj + per-partition bias."""
    for dt in range(DC):
        for (t0, tn) in chunks:
            ps = psB.tile([128, 2, 512], F32, tag="s", name="ps_p")
            for ct in range(n_ct):
                nc.tensor.matmul(
                    ps[:, 0, :tn],
                    w_sb[:, ct, dt * 128 : (dt + 1) * 128],
                    rhs[:, ct, t0 : t0 + tn],
                    start=(ct == 0),
                    stop=(ct == n_ct - 1),
                )
            nc.vector.tensor_scalar_add(
                out_sb[:, dt, t0 : t0 + tn], ps[:, 0, :tn], bias_sb[:, dt : dt + 1]
            )


def _attention(nc, P, prm, x_bf, x_all):
    psA, psB, qop, vp, expp, rzp = (
        P["psA"], P["psB"], P["qop"], P["vp"], P["expp"], P["rzp"]
    )
    ones_bf = P["ones_bf"]
    qT = qop.tile([128, DC, OWN], BF16, tag="qT", name="qT")
    _proj_to_T(nc, psB, prm["wq"], x_bf, qT, prm["bq"], QC)
    kT = P["kvp"].tile([128, DC, S], BF16, tag="kv", name="kT")
    _proj_to_T(
        nc, psB, prm["wk"], x_all, kT, prm["bk"], [(i * 394, 394) for i in range(4)]
    )

    v_bf = vp.tile([128, len(KT), NH, 128], BF16, tag="v", name="v_bf")
    for ti, (k0, ksz) in enumerate(KT):
        ps = psB.tile([128, 2, 512], F32, tag="s", name="ps_v")
        for half in range(2):
            for ct in range(DC):
                nc.tensor.matmul(
                    ps[:ksz, half, :512],
                    x_all[:, ct, k0 : k0 + ksz],
                    prm["wv"][:, ct, half * 512 : (half + 1) * 512],
                    start=(ct == 0),
                    stop=(ct == DC - 1),
                )
        nc.vector.tensor_tensor(
            v_bf[:ksz, ti, :, :],
            ps[:ksz, :, :].rearrange("t a b -> t (a b)").rearrange(
                "t (h e) -> t h e", h=NH
            ),
            prm["bv_bc"][:ksz, :].rearrange("t (h e) -> t h e", h=NH),
            OP.add,
        )

    oT = qop.tile([128, DC, OWN], BF16, tag="oT", name="oT")
    for hpair in range(NH // 2):
        hdt = hpair
        # P1: head even -> rows 0:64 = o_e, 64:128 = Z_e (ones-half of V')
        # P2: head odd  -> rows 0:64 = Z_o, 64:128 = o_o
        p1 = psA.tile([128, 2, 512], F32, tag="o", name="p1")
        p2 = psA.tile([128, 2, 512], F32, tag="zz", name="p2")
        for ti, (k0, ksz) in enumerate(KT):
            first, last = (ti == 0), (ti == len(KT) - 1)
            for sub in range(2):
                hp = sub * 64
                s_ps = psB.tile([128, 2, 512], F32, tag="s", name="s_ps")
                for ci, (q0, qn) in enumerate(QC):
                    nc.tensor.matmul(
                        s_ps[:ksz, ci, :qn],
                        kT[hp : hp + 64, hdt, k0 : k0 + ksz],
                        qT[hp : hp + 64, hdt, q0 : q0 + qn],
                        start=True,
                        stop=True,
                    )
                e_sb = expp.tile([128, 2, 394], BF16, tag="e", name="expS")
                nc.scalar.activation(
                    e_sb[:ksz, :, :], s_ps[:ksz, :, 0:394], AF.Exp, scale=0.125
                )
                dst = p1 if sub == 0 else p2
                for ci, (q0, qn) in enumerate(QC):
                    nc.tensor.matmul(
                        dst[:, ci, :qn],
                        v_bf[:ksz, ti, 2 * hpair + sub, :],
                        e_sb[:ksz, ci, :qn],
                        start=first,
                        stop=last,
                    )
        # 1/Z: Z_o in p2[0:64], Z_e in p1[64:128]; ln+exp base-aligned,
        # then swap halves with a f32r anti-diagonal-identity matmul.
        lnmix = rzp.tile([128, 2, 394], F32, tag="rz", name="lnmix")
        nc.scalar.activation(lnmix[0:64, :, :], p2[0:64, :, 0:394], AF.Ln)
        nc.scalar.activation(lnmix[64:128, :, :], p1[64:128, :, 0:394], AF.Ln)
        rzmix = rzp.tile([128, 2, 394], F32R, tag="rz", name="rzmix")
        nc.scalar.activation(rzmix[:], lnmix[:], AF.Exp, scale=-1.0)
        rsw_ps = psB.tile([128, 2, 512], F32, tag="s", name="rsw_ps")
        for ci in range(2):
            nc.tensor.matmul(
                rsw_ps[:, ci, :394],
                P["swap_sb"][:],
                rzmix[:, ci, :],
                start=True,
                stop=True,
            )
        rzs = rzp.tile([128, 2, 394], F32, tag="rz", name="rzs")
        nc.vector.tensor_scalar_add(rzs[:], rsw_ps[:, :, 0:394], 0.0)
        nc.vector.tensor_tensor(
            oT[0:64, hdt, :].rearrange("p (a b) -> p a b", a=2),
            p1[0:64, :, 0:394],
            rzs[0:64, :, :],
            OP.mult,
        )
        nc.vector.tensor_tensor(
            oT[64:128, hdt, :].rearrange("p (a b) -> p a b", a=2),
            p2[64:128, :, 0:394],
            rzs[64:128, :, :],
            OP.mult,
        )
    return oT


def _residual_proj(nc, psB, w_sb, rhs_T, bias_sb, x_bf, z_out):
    """z_out (f32) = W^T proj of rhs_T + bias + x_bf (residual)."""
    for dt in range(DC):
        for (q0, qn) in QC:
            ps = psB.tile([128, 2, 512], F32, tag="s", name="ps_r")
            for ct in range(DC):
                nc.tensor.matmul(
                    ps[:, 0, :qn],
                    w_sb[:, ct, dt * 128 : (dt + 1) * 128],
                    rhs_T[:, ct, q0 : q0 + qn],
                    start=(ct == 0),
                    stop=(ct == DC - 1),
                )
            nc.vector.scalar_tensor_tensor(
                z_out[:, dt, q0 : q0 + qn],
                ps[:, 0, :qn],
                bias_sb[:, dt : dt + 1],
                x_bf[:, dt, q0 : q0 + qn],
                OP.add,
                OP.add,
            )


def _ffn(nc, P, dr, l, prm, x_bf, z_out):
    psA, psB, bigp, wff = P["psA"], P["psB"], P["bigp"], P["wff"]
    h_bf = bigp.tile([128, FTC, OWN], BF16, tag="h", name="h_bf")
    for half in range(2):
        w1_sb = wff.tile([128, DC, FF // 2], BF16, tag="wff", name="w1_sb")
        nc.sync.dma_start(
            w1_sb[:],
            dr["w1T"][l][:, half * (FF // 2) : (half + 1) * (FF // 2)].rearrange(
                "(co p) f -> p co f", p=128
            ),
        )
        for j in range(FTC // 2):
            ft = half * (FTC // 2) + j
            ps = psB.tile([128, 2, 512], F32, tag="s", name="ps_h")
            for ci, (q0, qn) in enumerate(QC):
                for ct in range(DC):
                    nc.tensor.matmul(
                        ps[:, ci, :qn],
                        w1_sb[:, ct, j * 128 : (j + 1) * 128],
                        x_bf[:, ct, q0 : q0 + qn],
                        start=(ct == 0),
                        stop=(ct == DC - 1),
                    )
            nc.vector.tensor_scalar(
                h_bf[:, ft, :].rearrange("p (a b) -> p a b", a=2),
                ps[:, :, 0:394],
                prm["b1"][:, ft : ft + 1],
                0.0,
                OP.add,
                OP.max,
            )

    w2_halves = []
    for half in range(2):
        w2_sb = wff.tile([128, FTC // 2, D], BF16, tag="wff", name="w2_sb")
        nc.sync.dma_start(
            w2_sb[:],
            dr["w2T"][l][half * (FF // 2) : (half + 1) * (FF // 2), :].rearrange(
                "(fo p) d -> p fo d", p=128
            ),
        )
        w2_halves.append(w2_sb)
    for dt in range(DC):
        for (q0, qn) in QC:
            ps2 = psA.tile([128, 2, 512], F32, tag="o", name="ps_f")
            for half in range(2):
                for j in range(FTC // 2):
                    ft = half * (FTC // 2) + j
                    nc.tensor.matmul(
                        ps2[:, 0, :qn],
                        w2_halves[half][:, j, dt * 128 : (dt + 1) * 128],
                        h_bf[:, ft, q0 : q0 + qn],
                        start=(ft == 0),
                        stop=(ft == FTC - 1),
                    )
            nc.vector.scalar_tensor_tensor(
                z_out[:, dt, q0 : q0 + qn],
                ps2[:, 0, :qn],
                prm["b2"][:, dt : dt + 1],
                x_bf[:, dt, q0 : q0 + qn],
                OP.add,
                OP.add,
            )


def _one_layer(nc, P, dr, l, x_bf):
    prm = _load_layer_params(nc, P, dr, l)
    x_all = _allgather_x(nc, P, x_bf)
    oT = _attention(nc, P, prm, x_bf, x_all)
    z = P["zp"].tile([128, DC, OWN], F32R, tag="z", name=f"z1_{l}")
    _residual_proj(nc, P["psB"], prm["wo"], oT, prm["bo"], x_bf, z)
    x_bf = P["xp"].tile([128, DC, OWN], BF16, tag="x", name=f"x_ln1_{l}")
    _layernorm(nc, P, z, x_bf, prm["g1"], prm["be1"])
    z = P["zp"].tile([128, DC, OWN], F32R, tag="z", name=f"z2_{l}")
    _ffn(nc, P, dr, l, prm, x_bf, z)
    last = l == NL - 1
    if last:
        x2 = P["bigp"].tile([128, DC, OWN], F32, tag="h", name="x_final")
    else:
        x2 = P["xp"].tile([128, DC, OWN], BF16, tag="x", name=f"x_ln2_{l}")
    _layernorm(nc, P, z, x2, prm["g2"], prm["be2"])
    return x2


def _tail(nc, P, dr, x_f32):
    psB, bigp = P["psB"], P["bigp"]
    xout = dr["xout"]
    ident32 = P["constp"].tile([128, 128], F32, name="ident32")
    nc.vector.tensor_scalar_add(ident32[:], P["ident_sb"][:], 0.0)
    for ti in range(7):
        t0 = ti * 128
        tsz = min(128, OWN - t0)
        xo_sb = P["statp"].tile([128, D], F32, tag="st", name="xo_sb")
        for dt in range(DC):
            tp = psB.tile([128, 2, 512], F32, tag="s", name="tp")
            nc.tensor.transpose(
                tp[:tsz, 0, :128], x_f32[:, dt, t0 : t0 + tsz], ident32[:]
            )
            nc.vector.tensor_scalar_add(
                xo_sb[:tsz, dt * 128 : (dt + 1) * 128], tp[:tsz, 0, :128], 0.0
            )
        nc.sync.dma_start(xout[t0 : t0 + tsz, :], xo_sb[:tsz, :])


def _layernorm(nc, P, z, x_out, g_sb, be_sb):
    """Post-LN over features (partition dim) in transposed layout.

    z: [128, DC, OWN] f32r.  Writes x_out = (z - mu) * rstd * g + b.
    Stats are computed with f32r ones-matmuls directly on z (full precision
    vs the bf16 shadow it replaces)."""
    psB, statp, zbp = P["psB"], P["statp"], P["zbp"]
    ones_r = P["ones_r"]
    zf = z[:].bitcast(F32)
    sum_ps = psB.tile([128, 2, 512], F32, tag="s", name="sum_ps")
    for ci, (q0, qn) in enumerate(QC):
        for ct in range(DC):
            nc.tensor.matmul(
                sum_ps[:, ci, :qn],
                ones_r[:],
                z[:, ct, q0 : q0 + qn],
                start=(ct == 0),
                stop=(ct == DC - 1),
            )
    sq = zbp.tile([128, DC, OWN], F32R, tag="zb", name="sq_r")
    nc.vector.tensor_tensor(sq[:], zf, zf, OP.mult)
    sq_ps = psB.tile([128, 2, 512], F32, tag="s", name="sq_ps")
    for ci, (q0, qn) in enumerate(QC):
        for ct in range(DC):
            nc.tensor.matmul(
                sq_ps[:, ci, :qn],
                ones_r[:],
                sq[:, ct, q0 : q0 + qn],
                start=(ct == 0),
                stop=(ct == DC - 1),
            )
    mu = statp.tile([128, 2, 394], F32, tag="st", name="mu")
    nc.vector.tensor_scalar(
        mu[:], sum_ps[:, :, 0:394], 1.0 / D, None, OP.mult, OP.bypass
    )
    musq = statp.tile([128, 2, 394], F32, tag="st", name="musq")
    nc.vector.tensor_tensor(musq[:], mu[:], mu[:], OP.mult)
    var = statp.tile([128, 2, 394], F32, tag="st", name="var")
    nc.vector.scalar_tensor_tensor(
        var[:], sq_ps[:, :, 0:394], 1.0 / D, musq[:], OP.mult, OP.subtract
    )
    # rstd = exp(-0.5 * ln(var + eps))
    lnv = statp.tile([128, 2, 394], F32, tag="st", name="lnv")
    nc.scalar.activation(lnv[:], var[:], AF.Ln, bias=P["eps_sb"][:])
    rstd = statp.tile([128, 2, 394], F32, tag="st", name="rstd")
    nc.scalar.activation(rstd[:], lnv[:], AF.Exp, scale=-0.5)
    mr = statp.tile([128, 2, 394], F32, tag="st", name="mr")
    nc.vector.tensor_tensor(mr[:], mu[:], rstd[:], OP.mult)
    rstd_f = rstd[:].rearrange("p a b -> p (a b)")
    mr_f = mr[:].rearrange("p a b -> p (a b)")
    for ct in range(DC):
        nc.vector.tensor_tensor(z[:, ct, :], zf[:, ct, :], rstd_f[:, :788], OP.mult)
        nc.vector.tensor_tensor(z[:, ct, :], zf[:, ct, :], mr_f[:, :788], OP.subtract)
        nc.scalar.activation(
            x_out[:, ct, :],
            zf[:, ct, :],
            AF.Identity,
            bias=be_sb[:, ct : ct + 1],
            scale=g_sb[:, ct : ct + 1],
        )


def _build_wvx(Wv):
    """Extend Wv^T to [NL, D, NH*128]: per head a 64-col V block and a 64-col
    zero block (ones come from the bias); even heads [V|0], odd heads [0|V]."""
    bf = ml_dtypes.bfloat16
    WvT = Wv.transpose(0, 2, 1)  # [NL, D(c), D(v)]
    out = np.zeros((NL, D, NH * 128), np.float32)
    for h in range(NH):
        off = h * 128 + (0 if h % 2 == 0 else 64)
        out[:, :, off : off + 64] = WvT[:, :, h * 64 : (h + 1) * 64]
    return out.astype(bf)


def _build_bvx(bv):
    """Bias for the extended V: per head the V-half gets bv, ones-half gets 1."""
    out = np.ones((NL, NH * 128), np.float32)
    for h in range(NH):
        off = h * 128 + (0 if h % 2 == 0 else 64)
        out[:, off : off + 64] = bv[:, h * 64 : (h + 1) * 64]
    return out


_NC_CACHE = None


def _host_prep(inputs):
    """Patchify vid, build per-core inputs, pre-transpose weights (host-side)."""
    bf = ml_dtypes.bfloat16
    vid = np.asarray(inputs["vid"], np.float32)
    x = vid.reshape(B, L, C, H // PH, PH, W // PW, PW)
    x = x.transpose(0, 1, 3, 5, 4, 6, 2).reshape(B, L, NP, PD)

    pos = np.asarray(inputs["pos_emb"], np.float32)[0]  # [L, NP+1, D]
    cls = np.asarray(inputs["cls"], np.float32)[0, :, 0, :]  # [L, D]
    b_emb = np.asarray(inputs["b_embed"], np.float32)  # [D]

    shared = {
        "wembT": np.ascontiguousarray(
            np.asarray(inputs["W_embed"], np.float32).T
        ).astype(bf),
        "wqT": np.ascontiguousarray(
            np.asarray(inputs["Wq"], np.float32).transpose(0, 2, 1)
        ).astype(bf),
        "wkT": np.ascontiguousarray(
            np.asarray(inputs["Wk"], np.float32).transpose(0, 2, 1)
        ).astype(bf),
        "wvxT": _build_wvx(np.asarray(inputs["Wv"], np.float32)),
        "woT": np.ascontiguousarray(
            np.asarray(inputs["Wo"], np.float32).transpose(0, 2, 1)
        ).astype(bf),
        "w1T": np.ascontiguousarray(
            np.asarray(inputs["W1"], np.float32).transpose(0, 2, 1)
        ).astype(bf),
        "w2T": np.ascontiguousarray(
            np.asarray(inputs["W2"], np.float32).transpose(0, 2, 1)
        ).astype(bf),
        "bq": np.asarray(inputs["bq"], np.float32),
        "bk": np.asarray(inputs["bk"], np.float32),
        "bvx": _build_bvx(np.asarray(inputs["bv"], np.float32)),
        "bo": np.asarray(inputs["bo"], np.float32),
        "b1": np.asarray(inputs["b1"], np.float32),
        "b2": np.asarray(inputs["b2"], np.float32),
        "g1": np.asarray(inputs["ln1_g"], np.float32),
        "be1": np.asarray(inputs["ln1_b"], np.float32),
        "g2": np.asarray(inputs["ln2_g"], np.float32),
        "be2": np.asarray(inputs["ln2_b"], np.float32),
        "ident": np.eye(128, dtype=np.float32).astype(bf),
        "swapid": np.roll(np.eye(128, dtype=np.float32), 64, axis=1),
    }

    in_maps = []
    for c in range(N_CORES):
        b, half = c // 2, c % 2
        f0 = half * (L // 2)
        pat_c = np.zeros((PD, OWN), np.float32)
        addv_c = np.zeros((D, OWN), np.float32)
        for f in range(L // 2):
            fr = f0 + f
            t0 = f * (NP + 1)
            pat_c[:, t0 + 1 : t0 + NP + 1] = x[b, fr].T
            addv_c[:, t0] = pos[fr, 0] + cls[fr]
            addv_c[:, t0 + 1 : t0 + NP + 1] = (
                pos[fr, 1:].T + b_emb[:, None]
            )
        m = {"pat": pat_c.astype(bf), "addv": addv_c}
        m.update(shared)
        in_maps.append(m)
    return in_maps


def kernel(**inputs):
    global _NC_CACHE
    in_maps = _host_prep(inputs)
    if _NC_CACHE is None:
        nc = build_kernel()
        n = legalize_waits(nc)
        _NC_CACHE = nc
    nc = _NC_CACHE
    res = run_bass_kernel_spmd(nc, in_maps, core_ids=list(range(N_CORES)))
    out = np.zeros((B, S, D), np.float32)
    for c in range(N_CORES):
        b, half = c // 2, c % 2
        out[b, half * OWN : (half + 1) * OWN, :] = res.results[c]["xout"]
    return out
